# revision 20
# baseline (speedup 1.0000x reference)
"""Trainium2 Bass kernel for nn_ContiguousMatch.

Reference computation (per batch row b of x[B, L=30, A=21]):
    mv[b,l] = sum_a x[b,l,a] * v[l,a]          (V germline match prob)
    mj[b,l] = sum_a x[b,l,a] * j[l,a]          (J germline match prob)
    out[b]  = [ sum_l cumprod_l(mv[b,:]),      (expected match len from left)
                sum_l cumprod_l(mj[b,::-1]) ]  (expected match len from right)

Distribution: pure data parallel. x is sharded along batch across the
8 NeuronCores (50000 rows each, host-padded); the tiny [30,21]
germlines are baked into the program (they are one-hot, so the
per-position dot products are compile-time strided gathers).

Per-core dataflow (memory-bound: ~126 MB of x streamed at HBM rate):
  - batch rows are blocked per partition (each partition streams
    contiguous spans of rows), so every supertile DMA reads large
    contiguous spans per partition (~45KB descriptors -> near-peak HBM
    efficiency)
  - SDMA load balancing: a dma_start's descriptors (one per partition)
    are handed to the 16 SDMA engines in 8-descriptor chunks starting
    from engine 0, so a 128-partition DMA splits 8 descriptors per
    engine. SDMA engine 15 runs ~20% slower than engines 0-14 on this
    hardware (descriptor-ring port contention) and the whole stream
    waits for it. Mitigation: most supertiles are "base" ones covering
    all 128 partitions, and every ~5th supertile is an "extra" one over
    partitions 0-119 only (120 descriptors -> engines 0-14; engine 15
    gets none). Partitions 120-127 therefore carry DERATE15 (~0.82x)
    fewer batch rows, evening out per-engine finish times; every DMA
    keeps one large contiguous descriptor per partition. The extra rows
    live in a second DRAM region so both APs keep uniform partition
    strides.
    Under-filled partitions hold garbage in trailing supertile slots;
    it stays confined (S and R are fully rewritten each supertile, and
    garbage scan groups always trail the valid ones) and the host drops
    those columns.
  - per supertile, strided gather-copies pull m[b,l] = x[b,l,idx(l)]
    into a scan buffer laid out as groups of 31 slots (30 m-values +
    1 zero boundary slot); J groups hold mj reversed. The copies are
    split Vector/Scalar (~70ns vs ~192ns per instr) so both engines
    finish together.
  - one TensorTensorScanArith computes all cumprods in one pass:
        state = m[t]*state + d1[t]
    with m=0 / d1=1 at boundary slots resetting state to 1 between
    groups; a segmented reduce_sum -> [v_match, j_match] pairs
  - supertile sizes taper at the end ([12,9,6,5,4]) so the Vector
    engine is never backlogged when the final x bytes land; results
    accumulate in SBUF and are flushed in one early + one tiny final
    DMA; the host undoes the blocking.

If the germlines are ever NOT exactly one-hot, a general fallback path
computes the dot products with broadcast multiplies + segmented reduces.
"""

import math
import os
import sys

import numpy as np

for _p in ("/opt/trn_rl_repo",):
    if os.path.isdir(_p) and _p not in sys.path:
        sys.path.insert(0, _p)

import concourse.bacc as bacc
import concourse.mybir as mybir
import concourse.tile as tile
from contextlib import ExitStack


def _ensure_ntff_hook():
    """This image's ``antenv`` lacks ``axon_hooks``, which makes
    ``run_bass_kernel_spmd(trace=True)`` (or BASS_TRACE=1) crash on import.
    Recreate the tiny get/set module and register the ctypes NTFF hook from
    trn_agent_boot if available, so tracing works instead of crashing."""
    import types
    try:
        import antenv.axon_hooks  # noqa: F401
        return
    except ImportError:
        pass
    try:
        import antenv
    except ImportError:
        return
    mod = types.ModuleType("antenv.axon_hooks")
    mod._hook = None

    def set_axon_ntff_profile_hook(h):
        mod._hook = h

    def get_axon_ntff_profile_hook():
        return mod._hook

    mod.set_axon_ntff_profile_hook = set_axon_ntff_profile_hook
    mod.get_axon_ntff_profile_hook = get_axon_ntff_profile_hook
    sys.modules["antenv.axon_hooks"] = mod
    antenv.axon_hooks = mod
    try:
        from trn_agent_boot.trn_boot import _ntff_profile_via_ctypes
        so_path = "/opt/axon/libaxon_pjrt.so"
        if os.path.exists(so_path):
            mod._hook = _ntff_profile_via_ctypes(so_path)
    except Exception:
        pass


_ensure_ntff_hook()

B, L, A = 400000, 30, 21
LA = L * A
N_CORES = 8
P = 128
PF = 120    # partitions that also receive "extra" rows (engines 0-14)
GW = L + 1  # group width in the scan buffer: 30 m-values + 1 boundary slot
KMAX = 18   # max rows per partition per supertile
# Tapered final supertiles. Pieces stay >= 4 rows: a supertile costs
# ~2.6us of fixed Vector work (copies+scan+reduce) vs ~0.8us/row of DMA
# time, so tiles under ~4 rows add more compute backlog than stream time
# they cover and serialize after the last x byte lands.
TAIL = [12, 9, 6, 5, 4]
# Engine-15 row-count multiplier (see module docstring). On devices where
# SDMA engine 15 is the documented ~20%-slow runt AND ambient HBM bandwidth
# is not the binding cap, ~0.82 evens per-engine finish times. Measured
# today the chip-aggregate cap binds instead, so derating only shifts
# bytes onto engines 0-14 (+1.2%) for no gain: keep it off (1.0).
DERATE15 = 1.0
F32 = mybir.dt.float32

# Stash of the most recent BassKernelResults (test harness reads timing).
LAST_RESULTS = None
_PROG_CACHE = {}


def _layout(rows):
    """Returns (nf, nb): full-rate partitions (0..119) carry nf rows each,
    derated partitions (120..127) carry nb; nb/nf ~ DERATE15. Rows are laid
    out as region A = [128, nb] (base rows for every partition) followed by
    region B = [120, nf-nb] (extra rows for partitions 0-119);
    128*nb + 120*(nf-nb) >= rows, excess is host-side zero padding."""
    nf = int(math.ceil(rows / (PF + (P - PF) * DERATE15)))
    while True:
        nb = min(nf, int(round(nf * DERATE15)))
        if PF * nf + (P - PF) * nb >= rows:
            break
        nf += 1
    return nf, nb


def _sizes(h):
    """Supertile row counts summing to h: small front, KMAX middle,
    tapered tail."""
    if h <= KMAX:
        return [h]
    rem = h - sum(TAIL)
    if rem <= 0:
        ks = []
        while h > 0:
            ks.append(min(h, KMAX))
            h -= ks[-1]
        return ks
    n_full, r = divmod(rem, KMAX)
    if r == 0 and n_full >= 1:
        # split one full tile into a small front pair to fill the
        # DMA/compute pipeline quickly
        front = [KMAX // 3, KMAX - KMAX // 3]
        n_full -= 1
    else:
        front = [r] if r else []
    return front + [KMAX] * n_full + TAIL


def _plan(nf, nb):
    """Supertile plan: list of (kind, k) with kind 0 = base (all 128
    partitions, k rows each from region A) and kind 1 = extra (partitions
    0..PF-1 only, k rows each from region B). Extra supertiles are spread
    through the middle of the stream, away from the small-tile front and
    the tapered tail, so every DMA keeps large per-partition descriptors
    and engine 15 (which only serves base supertiles) streams its smaller
    share continuously."""
    ne = nf - nb
    ksb = _sizes(nb)
    sts = [(0, k) for k in ksb]
    if ne:
        kse = []
        left = ne
        while left > 0:
            kse.append(min(left, KMAX))
            left -= kse[-1]
        lo = min(2, len(sts))
        hi = max(lo, len(sts) - len(TAIL))
        span = hi - lo
        for i, k in enumerate(kse):
            pos = lo + (i + 1) * span // (len(kse) + 1) + i
            sts.insert(min(pos, len(sts) - len(TAIL)), (1, k))
    return sts


def _build_program(nf, nb, sts, v_idx=None, j_idx=None):
    """Build the per-core Bass program. If v_idx/j_idx are given, use the
    one-hot gather path; otherwise the general dot-product path with the
    germlines as runtime inputs."""
    gather = v_idx is not None
    kmax = max(k for _, k in sts)
    G = 2 * kmax  # scan groups per (max-size) supertile, interleaved (v, j)
    ne = nf - nb
    TOT = P * nb + PF * ne

    nc = bacc.Bacc("TRN2", target_bir_lowering=False, debug=False,
                   num_devices=N_CORES)
    x = nc.dram_tensor("x", [TOT, LA], F32, kind="ExternalInput").ap()
    # Output keeps the blocked layout [partition, 2*t + c] (one contiguous
    # store descriptor per partition); the host undoes the blocking.
    out = nc.dram_tensor("out", [P, 2 * nf], F32, kind="ExternalOutput").ap()
    if not gather:
        vg = nc.dram_tensor("vg", [L, A], F32, kind="ExternalInput").ap()
        jg = nc.dram_tensor("jg", [L, A], F32, kind="ExternalInput").ap()

    mult = mybir.AluOpType.mult

    # Region views: A = base rows (all partitions), B = extra rows (0..PF).
    xA = x[0:P * nb].rearrange("(p n) f -> p n f", p=P)
    xB = (x[P * nb:TOT].rearrange("(p n) f -> p n f", p=PF)
          if ne else None)

    with tile.TileContext(nc) as tc, ExitStack() as ctx:
        xpool = ctx.enter_context(tc.tile_pool(name="xin", bufs=4))
        cpool = ctx.enter_context(tc.tile_pool(name="const", bufs=1))

        M = cpool.tile([P, G * GW], F32)   # scan data0: m-values, 0 boundaries
        S = cpool.tile([P, G * GW], F32)   # scan output (cumprods)
        D1 = cpool.tile([P, G * GW], F32)  # scan data1: 1.0 at boundaries
        R = cpool.tile([P, 2 * nf], F32)

        nc.vector.memset(M[:, :], 0.0)
        nc.vector.memset(D1[:, :], 0.0)
        D13 = D1[:, :].rearrange("p (g c) -> p g c", c=GW)
        nc.vector.memset(D13[:, :, GW - 1], 1.0)

        M4 = M[:, :].rearrange("p (g two c) -> p g two c", two=2, c=GW)
        S3 = S[:, :].rearrange("p (g c) -> p g c", c=GW)

        if not gather:
            VB = cpool.tile([P, LA], F32)
            JB = cpool.tile([P, LA], F32)
            TMP = cpool.tile([P, kmax * LA], F32)
            # Broadcast the germlines to all 128 partitions during the DMA.
            nc.sync.dma_start(
                out=VB[:, :], in_=vg.flatten().unsqueeze(0).broadcast_to([P, LA]))
            nc.sync.dma_start(
                out=JB[:, :], in_=jg.flatten().unsqueeze(0).broadcast_to([P, LA]))

        def emit_m(xt3, kk):
            """Fill scan-buffer data slots from an x tile viewed [p, t, 630]."""
            m4 = M4[:, 0:kk]
            if gather:
                # Adjacent positions l, l+1 are gathered in one copy: their
                # two source columns sit at a compile-time-constant stride,
                # so a 2-element strided dim fetches both (halves the
                # instruction count vs one copy per position). A slice of
                # the V copies (always-positive src stride: 21 + idx delta)
                # runs on the otherwise-idle Activation engine in parallel
                # with the Vector engine; ACT copies cost ~192ns each vs
                # DVE's ~70ns, so ACT gets 8 of the 30 (8*192 ~ 22*70).
                for i, l in enumerate(range(0, L, 2)):
                    c0 = l * A + int(v_idx[l])
                    c1 = (l + 1) * A + int(v_idx[l + 1])
                    eng = nc.scalar.copy if i < 8 else nc.vector.tensor_copy
                    eng(m4[:, :, 0, l:l + 2],
                        xt3[:, :, c0::c1 - c0][:, :, 0:2])
                    # J slots are reversed: slot 29-l <- col of l. Ascending
                    # dst (28-l, 29-l) pairs with descending src (l+1, l).
                    d0 = (l + 1) * A + int(j_idx[l + 1])
                    d1 = l * A + int(j_idx[l])
                    nc.vector.tensor_copy(
                        m4[:, :, 1, L - 2 - l:L - l],
                        xt3[:, :, d0::d1 - d0][:, :, 0:2])
            else:
                t3 = TMP[:, 0:kk * LA].rearrange("p (t f) -> p t f", f=LA)
                t4 = TMP[:, 0:kk * LA].rearrange("p (t l a) -> p t l a",
                                                 l=L, a=A)
                for t in range(kk):
                    nc.vector.tensor_tensor(t3[:, t], xt3[:, t], VB[:, :], mult)
                nc.vector.reduce_sum(m4[:, :, 0, 0:L], t4,
                                     axis=mybir.AxisListType.X)
                for t in range(kk):
                    nc.vector.tensor_tensor(t3[:, t], xt3[:, t], JB[:, :], mult)
                nc.vector.reduce_sum(m4[:, :, 1, 0:L], t4[:, :, ::-1, :],
                                     axis=mybir.AxisListType.X)

        row = 0
        flushed = 0
        starts = []
        srcrow = [0, 0]  # next source row in region A / region B
        for i, (kind, kf) in enumerate(sts):
            starts.append(row)
            xt = xpool.tile([P, kmax * LA], F32, tag="xt")
            xt3 = xt[:, 0:kf * LA].rearrange("p (t f) -> p t f", f=LA)
            s0 = srcrow[kind]
            srcrow[kind] += kf
            if kind == 0:
                nc.sync.dma_start(out=xt3[:, 0:kf, :],
                                  in_=xA[:, s0:s0 + kf, :])
            else:
                nc.sync.dma_start(out=xt3[0:PF, 0:kf, :],
                                  in_=xB[:, s0:s0 + kf, :])
            if i == len(sts) - 2 and i >= 3:
                # Flush all result columns finished a few supertiles ago.
                # Placed AFTER this supertile's x-DMA issues and lagging far
                # enough that its wait (on an old reduce) is already
                # satisfied, so it never stalls the x-stream FIFO. Only the
                # last supertiles' few columns stay on the critical path.
                flushed = starts[i - 2]
                nc.sync.dma_start(out=out[:, 0:2 * flushed],
                                  in_=R[:, 0:2 * flushed])
            emit_m(xt3, kf)
            nc.vector.tensor_tensor_scan(
                S[:, 0:2 * kf * GW], M[:, 0:2 * kf * GW], D1[:, 0:2 * kf * GW],
                1.0, mult, mybir.AluOpType.add)
            nc.vector.reduce_sum(R[:, 2 * row:2 * (row + kf)],
                                 S3[:, 0:2 * kf, 0:L],
                                 axis=mybir.AxisListType.X)
            row += kf
        assert row == nf and srcrow == [nb, ne], (row, srcrow)
        nc.sync.dma_start(out=out[:, 2 * flushed:], in_=R[:, 2 * flushed:])

    nc.compile()
    return nc


def _get_program(nf, nb, sts, v, j):
    """Return (nc, gather) with compile-spec caching."""
    v_idx = v.argmax(axis=1)
    j_idx = j.argmax(axis=1)
    vh = np.zeros_like(v)
    vh[np.arange(L), v_idx] = 1.0
    jh = np.zeros_like(j)
    jh[np.arange(L), j_idx] = 1.0
    gather = np.array_equal(v, vh) and np.array_equal(j, jh)
    spec = (nf, nb, tuple(sts))
    if gather:
        key = spec + ("gather", tuple(int(i) for i in v_idx),
                      tuple(int(i) for i in j_idx))
    else:
        key = spec + ("general",)
    if key not in _PROG_CACHE:
        if gather:
            _PROG_CACHE[key] = (_build_program(nf, nb, sts, v_idx, j_idx),
                                True)
        else:
            _PROG_CACHE[key] = (_build_program(nf, nb, sts), False)
    return _PROG_CACHE[key]


def kernel(x, v_germline_aa_onehot, j_germline_aa_onehot):
    global LAST_RESULTS
    from concourse.bass_utils import run_bass_kernel_spmd

    x = np.asarray(x, dtype=np.float32)
    v = np.ascontiguousarray(np.asarray(v_germline_aa_onehot, dtype=np.float32))
    j = np.ascontiguousarray(np.asarray(j_germline_aa_onehot, dtype=np.float32))
    Bt = x.shape[0]
    assert Bt % N_CORES == 0, Bt
    rows = Bt // N_CORES            # 50000
    nf, nb = _layout(rows)
    ne = nf - nb
    sts = _plan(nf, nb)
    TOT = P * nb + PF * ne

    nc, gather = _get_program(nf, nb, sts, v, j)

    xr = np.ascontiguousarray(x).reshape(Bt, LA)
    in_maps = []
    for c in range(N_CORES):
        shard = xr[c * rows:(c + 1) * rows]
        if TOT != rows:
            shard = np.concatenate(
                [shard, np.zeros((TOT - rows, LA), np.float32)], axis=0)
        m = {"x": shard}
        if not gather:
            m["vg"] = v
            m["jg"] = j
        in_maps.append(m)

    res = run_bass_kernel_spmd(nc, in_maps, core_ids=list(range(N_CORES)))
    LAST_RESULTS = res

    # Undo the blocked layout back to batch-major [rows, 2] per core.
    # Partition p holds base rows [p*nb, (p+1)*nb) (region A) and, for
    # p < PF, extra rows [P*nb + p*ne, ...+ne) (region B). Base supertiles
    # fill the next cb result-column pairs with region-A rows in order;
    # extra supertiles do the same with region-B rows.
    C = np.cumsum([0] + [k for _, k in sts])
    colsA = np.concatenate(
        [np.arange(C[i], C[i] + k)
         for i, (kind, k) in enumerate(sts) if kind == 0])
    colsB = (np.concatenate(
        [np.arange(C[i], C[i] + k)
         for i, (kind, k) in enumerate(sts) if kind == 1])
        if ne else np.empty(0, np.int64))
    assert len(colsA) == nb and len(colsB) == ne
    final = np.empty((Bt, 2), np.float32)
    for c in range(N_CORES):
        r = res.results[c]["out"].reshape(P, nf, 2)
        # region A rows, clipped to the unpadded row count
        nA = min(P * nb, rows)
        final[c * rows:c * rows + nA] = (
            r[:, colsA, :].reshape(P * nb, 2)[:nA])
        # region B: valid prefix up to `rows`
        if ne:
            nB = rows - P * nb
            rb = r[:PF, colsB, :].reshape(PF * ne, 2)
            final[c * rows + nA:(c + 1) * rows] = rb[:nB]
    return final
